# revision 26
# baseline (speedup 1.0000x reference)
"""3-layer GAT on 8 trn2 NeuronCores (Bass/Tile).

Strategy (dst-node sharding):
- N padded to 50176 = 392 dst-blocks x 128; 49 blocks per core.
- Per layer, a node-feature table holds per-node rows [h bf16 | el f32 | er f32]
  (768 B rows for 256-dim layers, 256 B rows for the 64-dim layer-3 input).
- Table rows use a chunk-interleaved layout: local blocks are grouped into
  chunks of CHK=7; chunk k of all 8 cores is contiguous (core-major inside
  the chunk).  That makes each chunk's AllGather output contiguous, so the
  three table AllGathers are split into 7 chunk-AGs fired as soon as their
  shard rows are written -- they overlap the dense/edge compute instead of
  serializing between layers.
- Dense phase is sharded: each core computes rows for its 6272 nodes (fp32
  matmuls on PE), writing its shard; chunk-AGs replicate to every core.
- Edge phase per dst-block: dma_gather pulls h|el rows by (remapped) src
  (two gathers, trow < 32768 and >= 32768, because gather indices are
  int16).  Gather descriptor generation on gpsimd is the kernel bottleneck
  (~8 ns/row), so the index arrays are -1-padded and the true per-block
  counts are loaded into gpsimd registers at runtime (reg_load) so the
  ucode only generates descriptors for real edges.
- er expansion per edge uses a host-precomputed transposed one-hot selT
  (bf16, streamed per block from HBM) as matmul lhsT -- no PE transposes or
  PSUM round-trips.  The dst one-hot sel is built on DVE and used to
  segment-reduce [w | w*h] into PSUM over the block's edge tiles, yielding
  softmax denominator and weighted sum together: out = (sum w*h)/(sum w).
- The next layer's dense matmul for the block's 128 nodes is interleaved
  right after each block epilogue so it hides inside the gather stream.
"""

import os
import sys

sys.path.insert(0, "/opt/trn_rl_repo")

PHASES = int(os.environ.get("GAT_PHASES", "99"))
EDGE_CUT = int(os.environ.get("GAT_EDGE_CUT", "99"))
EXACT = int(os.environ.get("GAT_EXACT", "0"))  # -1-padded gathers + runtime counts
NOCHUNK = int(os.environ.get("GAT_NOCHUNK", "0"))  # AllGathers after loops, no overlap
SINGLE_PACKET = bool(int(os.environ.get("GAT_SP", "0")))  # dma_gather single_packet

import numpy as np
import ml_dtypes

import concourse.bass as bass
import concourse.tile as tile
import concourse.mybir as mybir
from concourse import bacc
from concourse.bass_utils import run_bass_kernel_spmd

bf16 = mybir.dt.bfloat16
f32 = mybir.dt.float32
i16 = mybir.dt.int16
i32 = mybir.dt.int32
AF = mybir.ActivationFunctionType
ALU = mybir.AluOpType

NCORES = 8
P = 128
SPLIT = 32768
NEG_SLOPE = 0.2
H = 4
F = 64
D = H * F  # 256
ROW = 384  # bf16 cols per 256-dim table row (h 0:256 | el f32 256:264 | er f32 264:272 | pad)
ROW3 = 128  # bf16 cols per 64-dim table row (h 0:64 | el f32 64:66 | er f32 66:68 | pad)
CHK = 7  # blocks per AllGather chunk


def _wrap_idx_blocks(arr):
    """[NBLK, K] int16 -> [128, NBLK*K//16] in dma_gather index layout
    (idx i of each block at partition i%16, col i//16; 16-row pattern tiled
    8x down the partitions)."""
    nblk, k = arr.shape
    a = arr.reshape(nblk, k // 16, 16).transpose(0, 2, 1)  # [NBLK, 16, K/16]
    a = np.tile(a, (1, 8, 1))  # [NBLK, 128, K/16]
    return np.ascontiguousarray(a.transpose(1, 0, 2).reshape(128, -1))


def _col_layout(arr):
    """[NBLK, T*128] -> [128, NBLK*T]: slot t*128+p of block b at
    (p, b*T + t) -- matches the gather tile layout."""
    nblk, tk = arr.shape
    t = tk // 128
    a = arr.reshape(nblk, t, 128).transpose(2, 0, 1)  # [128, NBLK, T]
    return np.ascontiguousarray(a.reshape(128, nblk * t))


def _block_diag(a):
    """[H, F] -> [H*F, H] with a[h] on block-column h."""
    h, f = a.shape
    out = np.zeros((h * f, h), np.float32)
    for i in range(h):
        out[i * f : (i + 1) * f, i] = a[i]
    return out


def kernel(feat, src, dst, W1, al1, ar1, b1, W2, al2, ar2, b2, W3, al3, ar3, b3):
    feat = np.asarray(feat, np.float32)
    src = np.asarray(src).astype(np.int64)
    dst = np.asarray(dst).astype(np.int64)
    params = [np.asarray(p, np.float32) for p in (W1, al1, ar1, b1, W2, al2, ar2, b2, W3, al3, ar3, b3)]
    W1, al1, ar1, b1, W2, al2, ar2, b2, W3, al3, ar3, b3 = params
    assert abs(b1).max() == 0 and abs(b2).max() == 0 and abs(b3).max() == 0, (
        "non-zero GAT biases not implemented"
    )

    N, DIN = feat.shape
    E = src.shape[0]
    nblk_raw = -(-N // P)
    NBLK = -(-nblk_raw // NCORES) * NCORES  # 392
    NPAD = NBLK * P  # 50176
    BPC = NBLK // NCORES  # 49
    SHARD = BPC * P  # 6272
    # AllGather chunks over local blocks; small final chunk so the exposed
    # AG tail at each layer transition is tiny.
    CH = [8, 8, 8, 8, 8, 8, 1]
    assert sum(CH) == BPC
    NCHUNK = len(CH)
    CS = np.concatenate([[0], np.cumsum(CH)])  # chunk start blocks, len NCHUNK+1

    # ---- host: edge preprocessing ----
    # table-row remap: node (core c, local block b, lane p) lives at
    # trow = 8*128*CS[k] + c*(CH[k]*128) + (b-CS[k])*128 + p  (k = chunk of b)
    # so each chunk's AllGather output is contiguous (core-major in-chunk).
    def trow_of(node):
        c = node // SHARD
        r = node % SHARD
        b = r // P
        p = r % P
        k = np.searchsorted(CS, b, side="right") - 1
        return NCORES * P * CS[k] + c * (np.asarray(CH)[k] * P) + (b - CS[k]) * P + p

    blk = dst // P
    order = np.lexsort((src, blk))
    tsrc_s = trow_of(src[order])
    dloc_s = (dst - blk * P)[order]
    blk_s = blk[order]
    counts = np.bincount(blk_s, minlength=NBLK)
    bstart = np.zeros(NBLK + 1, np.int64)
    np.cumsum(counts, out=bstart[1:])

    nlo = np.empty(NBLK, np.int64)
    nhi = np.empty(NBLK, np.int64)
    lo_list = []
    hi_list = []
    dlo_list = []
    dhi_list = []
    for b in range(NBLK):
        s, e = bstart[b], bstart[b + 1]
        ts = tsrc_s[s:e]
        dd = dloc_s[s:e]
        m = ts < SPLIT
        lo_list.append(ts[m])
        hi_list.append(ts[~m] - SPLIT)
        dlo_list.append(dd[m])
        dhi_list.append(dd[~m])
        nlo[b] = int(m.sum())
        nhi[b] = int((e - s) - nlo[b])

    # per-LOCAL-block gather sizes: max over the 8 cores (SPMD shares the
    # program, but block j's K need only cover the 8 cores' counts, not the
    # global max) -- saves ~10% of gather descriptors.
    nlo_c = nlo.reshape(NCORES, BPC)
    nhi_c = nhi.reshape(NCORES, BPC)
    KLO_J = (-(-nlo_c.max(axis=0) // P) * P).astype(np.int64)  # [BPC]
    KHI_J = (-(-nhi_c.max(axis=0) // P) * P).astype(np.int64)
    TLO_J = KLO_J // P
    T_J = (KLO_J + KHI_J) // P
    TM = int(T_J.max())
    LO_OFF = np.concatenate([[0], np.cumsum(KLO_J // 16)])
    HI_OFF = np.concatenate([[0], np.cumsum(KHI_J // 16)])
    DL_OFF = np.concatenate([[0], np.cumsum(T_J)])
    SELT_OFF = np.concatenate([[0], np.cumsum(T_J * P)])

    pad_idx = -1 if EXACT else 0
    lo_idx = []  # per global block, [K_LO_j] int16
    hi_idx = []
    dstloc = []  # per global block, [T_j*128] f32
    for b in range(NBLK):
        j = b % BPC
        nl, nh = int(nlo[b]), int(nhi[b])
        li = np.full(int(KLO_J[j]), pad_idx, np.int16)
        hi_ = np.full(int(KHI_J[j]), pad_idx, np.int16)
        dl = np.full(int(T_J[j]) * P, -1.0, np.float32)
        li[:nl] = lo_list[b]
        hi_[:nh] = hi_list[b]
        dl[:nl] = dlo_list[b]
        dl[int(KLO_J[j]) : int(KLO_J[j]) + nh] = dhi_list[b]
        lo_idx.append(li)
        hi_idx.append(hi_)
        dstloc.append(dl)

    cnts = np.stack([nlo, nhi], axis=1).astype(np.int32)  # [NBLK, 2]

    # ---- host: weights ----
    def wall(W, al, ar):
        wel = W @ _block_diag(al)
        wer = W @ _block_diag(ar)
        return np.concatenate([W, wel, wer], axis=1).astype(np.float32)

    wall1 = wall(W1, al1, ar1)  # [DIN, 264]
    wall2 = wall(W2, al2, ar2)  # [256, 264]
    wall3 = wall(W3, al3, ar3)  # [256, 66]
    NW = D + 2 * H  # 264
    NW3 = F + 2  # 66

    featT = np.zeros((DIN, NPAD), np.float32)
    featT[:, :N] = feat.T

    iota_np = np.tile(np.arange(P, dtype=np.float32), (P, 1)).astype(ml_dtypes.bfloat16)
    idn_np = np.eye(P, dtype=np.float32)

    # ---- host: per-core const blob (single int16 tensor -> one DMA) ----
    wall1b = wall1.astype(ml_dtypes.bfloat16)  # bf16 copy for fast dense-1

    def blob_for_core(c):
        b0 = c * BPC
        fields = [
            iota_np.view(np.int16),  # 128 cols bf16
            idn_np.view(np.int16),  # 256 cols f32
            wall1b.view(np.int16),  # [DIN, 264] bf16
            wall2[0:P].view(np.int16),
            wall2[P : 2 * P].view(np.int16),
            wall3[0:P].view(np.int16),
            wall3[P : 2 * P].view(np.int16),
        ]
        fields += [_wrap_idx_blocks(lo_idx[b0 + j][None, :]) for j in range(BPC)]
        fields += [_wrap_idx_blocks(hi_idx[b0 + j][None, :]) for j in range(BPC)]
        fields += [np.tile(cnts[b0 : b0 + BPC].view(np.int16).reshape(1, -1), (P, 1))]
        fields += [
            _col_layout(dstloc[b0 + j].astype(ml_dtypes.bfloat16).view(np.int16)[None, :])
            for j in range(BPC)
        ]
        for f_ in fields:
            assert f_.shape[0] == P, f_.shape
        blob = np.concatenate(fields, axis=1)
        if blob.shape[1] % 2:
            blob = np.concatenate([blob, np.zeros((P, 1), np.int16)], axis=1)
        return np.ascontiguousarray(blob)

    def selt_for_core(c):
        """Transposed dst one-hot: [128 (dstnode q), sum_j T_j*128] bf16
        viewed as int16; col (j, t*128+p) is 1.0 iff dstloc[j][t*128+p]==q."""
        b0 = c * BPC
        d = np.concatenate([dstloc[b0 + j] for j in range(BPC)]).reshape(1, -1)
        q = np.arange(P, dtype=np.float32).reshape(P, 1)
        sel = (d == q).astype(ml_dtypes.bfloat16)
        return np.ascontiguousarray(sel.view(np.int16))

    assert DIN == P, "layer-1 input dim must be 128"
    offs = {}
    o = 0
    for name, w in [
        ("iota", 128),
        ("idn", 256),
        ("wall1b", NW),
        ("wall2k0", 2 * NW),
        ("wall2k1", 2 * NW),
        ("wall3k0", 2 * NW3),
        ("wall3k1", 2 * NW3),
        ("lo", int(LO_OFF[-1])),
        ("hi", int(HI_OFF[-1])),
        ("cnts", BPC * 4),
        ("dstloc", int(DL_OFF[-1])),
    ]:
        offs[name] = o
        o += w
    blob0 = blob_for_core(0)
    CB = blob0.shape[1]
    assert o == CB or o + 1 == CB, (o, CB)
    assert offs["cnts"] % 2 == 0, "int32 counts need even int16 offset"
    NSELT = int(SELT_OFF[-1])

    # ---- build program (identical for all cores; per-core data via inputs) ----
    nc = bacc.Bacc("TRN2", target_bir_lowering=False, debug=False, num_devices=NCORES)

    cblob_in = nc.dram_tensor("cblob", [P, CB], i16, kind="ExternalInput")
    featT_in = nc.dram_tensor("featT", [P, SHARD], f32, kind="ExternalInput")
    selt_in = nc.dram_tensor("selt", [P, NSELT], i16, kind="ExternalInput")
    out_ext = nc.dram_tensor("out", [SHARD, F], f32, kind="ExternalOutput")

    tab1_sh = nc.dram_tensor("tab1_sh", [SHARD, ROW], bf16)
    tab2_sh = nc.dram_tensor("tab2_sh", [SHARD, ROW], bf16)
    tab3_sh = nc.dram_tensor("tab3_sh", [SHARD, ROW3], bf16)
    tab1 = nc.dram_tensor("tab1", [NPAD, ROW], bf16, addr_space="Shared")
    tab2 = nc.dram_tensor("tab2", [NPAD, ROW], bf16, addr_space="Shared")
    tab3 = nc.dram_tensor("tab3", [NPAD, ROW3], bf16, addr_space="Shared")

    rg = [list(range(NCORES))]

    with tile.TileContext(nc) as tc:
        with (
            tc.tile_pool(name="const", bufs=1) as cp,
            tc.tile_pool(name="work", bufs=3) as wp,
            tc.tile_pool(name="small", bufs=3) as sp,
            tc.tile_pool(name="psum", bufs=2, space="PSUM") as pp,
        ):
            cblob = cp.tile([P, CB], i16)
            nc.sync.dma_start(cblob[:], cblob_in[:])
            iota = cblob[:, offs["iota"] : offs["iota"] + 128].bitcast(bf16)
            idn = cblob[:, offs["idn"] : offs["idn"] + 256].bitcast(f32)
            wall1_t = cblob[:, offs["wall1b"] : offs["wall1b"] + NW].bitcast(bf16)
            wall2_t = [
                cblob[:, offs[f"wall2k{k}"] : offs[f"wall2k{k}"] + 2 * NW].bitcast(f32)
                for k in range(2)
            ]
            wall3_t = [
                cblob[:, offs[f"wall3k{k}"] : offs[f"wall3k{k}"] + 2 * NW3].bitcast(f32)
                for k in range(2)
            ]

            rcnt_lo = nc.gpsimd.alloc_register("cnt_lo")
            rcnt_hi = nc.gpsimd.alloc_register("cnt_hi")
            kregs = {
                int(v): nc.gpsimd.to_reg(int(v))
                for v in sorted(set(KLO_J.tolist()) | set(KHI_J.tolist()))
            }

            def idx_ap(field, joff, k16):
                off = offs[field] + joff
                return cblob[:, off : off + k16]

            def cnt_ap(j, which):
                off = offs["cnts"] + j * 4 + which * 2
                return cblob[0:1, off : off + 2].bitcast(i32)

            CHUNK_END = {int(CS[k + 1]) - 1: k for k in range(NCHUNK)}

            def ag_chunk_one(src_t, dst_t, k):
                lo, hi = int(CS[k]) * P, int(CS[k + 1]) * P
                nc.gpsimd.collective_compute(
                    "AllGather",
                    ALU.bypass,
                    replica_groups=rg,
                    ins=[src_t[lo:hi]],
                    outs=[dst_t[NCORES * lo : NCORES * hi]],
                )

            def ag_chunk(src_t, dst_t, k):
                if NOCHUNK:
                    if k == NCHUNK - 1:  # all chunks after the loop, no overlap
                        for kk in range(NCHUNK):
                            ag_chunk_one(src_t, dst_t, kk)
                    return
                ag_chunk_one(src_t, dst_t, k)

            def dense_write(x_ap, j, wall_k, nw, tab_shard, row_cols, hsz, first):
                """dense for 128 nodes of block j: rows [h bf16 | el er f32]
                written to tab_shard. x_ap: [128, 256] f32 node-major (SBUF),
                or None with `first` giving the layer-1 lhsT directly."""
                psd = pp.tile([P, NW], f32, tag="psd", space="PSUM")
                nk = len(wall_k)
                if first is not None:
                    nc.tensor.matmul(psd[:, :nw], first, wall_k[0][:, :nw], start=True, stop=True)
                else:
                    lhsT = sp.tile([P, 2, P], f32, tag="lhsT")
                    for k in range(nk):
                        ptr = pp.tile([P, P], f32, tag="ptr", space="PSUM")
                        nc.tensor.transpose(ptr[:], x_ap[:, k * P : (k + 1) * P], idn)
                        nc.vector.tensor_copy(lhsT[:, k, :], ptr[:])
                    for k in range(nk):
                        nc.tensor.matmul(
                            psd[:, :nw],
                            lhsT[:, k, :],
                            wall_k[k][:, :nw],
                            start=(k == 0),
                            stop=(k == nk - 1),
                        )
                row = sp.tile([P, row_cols], bf16, tag="row")
                nc.vector.tensor_copy(row[:, 0:hsz], psd[:, 0:hsz])
                nc.vector.tensor_copy(
                    row[:, hsz : hsz + 2 * (nw - hsz)].bitcast(f32),
                    psd[:, hsz:nw],
                )
                nc.sync.dma_start(tab_shard[j * P : (j + 1) * P, :], row[:])

            def dump_rows(tab_shard, row, hsz):
                """debug: write first 64 h-cols of each shard row to out_ext"""
                for j in range(BPC):
                    r = sp.tile([P, row], bf16, tag="dump")
                    nc.sync.dma_start(r[:], tab_shard[j * P : (j + 1) * P, :])
                    rf = sp.tile([P, F], f32, tag="dumpf")
                    nc.vector.tensor_copy(rf[:], r[:, 0:F])
                    nc.sync.dma_start(out_ext[j * P : (j + 1) * P, :], rf[:])

            # ---- dense layer 1 (sharded; lhsT = feat^T slices, K=128, bf16) ----
            for j in range(BPC):
                ft = sp.tile([P, P], f32, tag="ft")
                nc.sync.dma_start(ft[:], featT_in[:, j * P : (j + 1) * P])
                ftb = sp.tile([P, P], bf16, tag="ftb")
                nc.vector.tensor_copy(ftb[:], ft[:])
                dense_write(None, j, [wall1_t], NW, tab1_sh, ROW, D, first=ftb[:])
                if PHASES >= 2 and j in CHUNK_END:
                    ag_chunk(tab1_sh, tab1, CHUNK_END[j])

            if PHASES == 1:
                dump_rows(tab1_sh, ROW, D)

            # ---- edge phase for one layer ----
            def edge_layer(tab_full, tab_shard, row, heads, hsz, nxt, next_ag):
                """tab_full: AG'd table, tab_shard: local shard (er source),
                row: bf16 cols per table row, heads: H, hsz: h cols,
                nxt: (wall_k, nw, tab_shard_next, row_next, hsz_next) or
                'out' for the final layer.  next_ag: (shard_t, full_t) to
                chunk-AllGather as next-layer rows complete, or None."""
                nmsg = heads + hsz
                # zero the hx ring so stale SBUF bits can't turn into
                # NaN/inf messages on slots the shortened gathers skip
                for _ in range(3):
                    hx0 = wp.tile([P, TM, row], bf16, tag="hx")
                    nc.gpsimd.memset(hx0[:].rearrange("p t r -> p (t r)"), 0.0)
                for j in range(BPC):
                    klo, khi = int(KLO_J[j]), int(KHI_J[j])
                    tlo, tj = int(TLO_J[j]), int(T_J[j])
                    hx = wp.tile([P, TM, row], bf16, tag="hx")
                    if EXACT:
                        nc.gpsimd.reg_load(rcnt_lo, cnt_ap(j, 0))
                    nc.gpsimd.dma_gather(
                        hx[:, 0:tlo, :],
                        tab_full[0:SPLIT],
                        idx_ap("lo", int(LO_OFF[j]), klo // 16),
                        klo,
                        rcnt_lo if EXACT else kregs[klo],
                        row,
                        elem_step=row,
                        single_packet=SINGLE_PACKET,
                    )
                    if EXACT:
                        nc.gpsimd.reg_load(rcnt_hi, cnt_ap(j, 1))
                    nc.gpsimd.dma_gather(
                        hx[:, tlo:tj, :],
                        tab_full[SPLIT:NPAD],
                        idx_ap("hi", int(HI_OFF[j]), khi // 16),
                        khi,
                        rcnt_hi if EXACT else kregs[khi],
                        row,
                        elem_step=row,
                        single_packet=SINGLE_PACKET,
                    )
                    # er for the block's 128 dsts: direct strided load of the
                    # 256B [el|er] row chunk, cast er to bf16
                    erch = sp.tile([P, 128], bf16, tag="erch")
                    nc.sync.dma_start(
                        erch[:], tab_shard[j * P : (j + 1) * P, row - 128 : row]
                    )
                    eroff0 = 128 - (row - hsz)
                    er_blk = sp.tile([P, heads], bf16, tag="er_blk")
                    nc.scalar.activation(
                        er_blk[:],
                        erch[:, eroff0 + 2 * heads : eroff0 + 4 * heads].bitcast(f32),
                        AF.Copy,
                    )
                    if EDGE_CUT == 1:
                        # dump gathered h cols 0:64 of tile 0
                        df = sp.tile([P, F], f32, tag="edump")
                        nc.vector.tensor_copy(df[:], hx[:, 0, 0:F])
                        nc.sync.dma_start(out_ext[j * P : (j + 1) * P, :], df[:])
                        continue
                    # per-edge er via host-precomputed transposed one-hot
                    selt_sb = wp.tile([P, TM * P], i16, tag="selt")
                    so = int(SELT_OFF[j])
                    nc.sync.dma_start(
                        selt_sb[:, 0 : tj * P], selt_in[:, so : so + tj * P]
                    )
                    er_ps = pp.tile([P, TM * heads], f32, tag="erps", space="PSUM")
                    for t in range(tj):
                        nc.tensor.matmul(
                            er_ps[:, t * heads : (t + 1) * heads],
                            selt_sb[:, t * P : (t + 1) * P].bitcast(bf16),
                            er_blk[:],
                            start=True,
                            stop=True,
                        )
                    # dst one-hot for the segment matmul (DVE)
                    sel = wp.tile([P, TM, P], bf16, tag="sel")
                    dl_off = offs["dstloc"] + int(DL_OFF[j])
                    nc.vector.tensor_tensor(
                        out=sel[:, 0:tj, :],
                        in0=cblob[:, dl_off : dl_off + tj]
                        .bitcast(bf16)
                        .unsqueeze(2)
                        .to_broadcast([P, tj, P]),
                        in1=iota.unsqueeze(1).to_broadcast([P, tj, P]),
                        op=ALU.is_equal,
                    )
                    # e = el[src] + er[dst]; w = exp(lrelu(e))
                    el_src = hx[:, 0:tj, hsz : hsz + 2 * heads].bitcast(f32)
                    e_t = sp.tile([P, TM, heads], f32, tag="e_t")
                    nc.vector.tensor_tensor(
                        out=e_t[:, 0:tj, :],
                        in0=el_src,
                        in1=er_ps[:, 0 : tj * heads].rearrange(
                            "p (t h) -> p t h", h=heads
                        ),
                        op=ALU.add,
                    )
                    lr = sp.tile([P, TM, heads], f32, tag="lr")
                    nc.vector.tensor_scalar_mul(lr[:, 0:tj, :], e_t[:, 0:tj, :], NEG_SLOPE)
                    nc.vector.tensor_tensor(
                        out=lr[:, 0:tj, :], in0=e_t[:, 0:tj, :], in1=lr[:, 0:tj, :],
                        op=ALU.max,
                    )
                    msg = wp.tile([P, TM, nmsg], bf16, tag="msg")
                    nc.scalar.activation(msg[:, 0:tj, 0:heads], lr[:, 0:tj, :], AF.Exp)
                    # wh = w * h
                    nc.vector.tensor_tensor(
                        out=msg[:, 0:tj, heads:nmsg],
                        in0=hx[:, 0:tj, 0:hsz],
                        in1=msg[:, 0:tj, 0:heads]
                        .unsqueeze(3)
                        .to_broadcast([P, tj, heads, F]),
                        op=ALU.mult,
                    )
                    if EDGE_CUT == 2:
                        df = sp.tile([P, F], f32, tag="edump")
                        nc.vector.tensor_copy(df[:], msg[:, 0, heads : heads + F])
                        nc.sync.dma_start(out_ext[j * P : (j + 1) * P, :], df[:])
                        continue
                    # segment-reduce into PSUM
                    ps = pp.tile([P, nmsg], f32, tag="agg", space="PSUM")
                    for t in range(tj):
                        nc.tensor.matmul(
                            ps[:],
                            sel[:, t, :],
                            msg[:, t, :],
                            start=(t == 0),
                            stop=(t == tj - 1),
                        )
                    if EDGE_CUT == 3:
                        df = sp.tile([P, F], f32, tag="edump")
                        nc.vector.tensor_copy(df[:], ps[:, heads : heads + F])
                        nc.sync.dma_start(out_ext[j * P : (j + 1) * P, :], df[:])
                        continue
                    if EDGE_CUT == 4:
                        df = sp.tile([P, F], f32, tag="edump")
                        nc.gpsimd.memset(df[:], 0.0)
                        nc.vector.tensor_copy(df[:, 0:heads], ps[:, 0:heads])
                        nc.sync.dma_start(out_ext[j * P : (j + 1) * P, :], df[:])
                        continue
                    # epilogue: out = act(wh_sum / w_sum)
                    rcp = sp.tile([P, 2, heads], f32, tag="rcp")
                    nc.vector.tensor_scalar(
                        out=rcp[:, 0, :], in0=ps[:, 0:heads], scalar1=1e-30,
                        scalar2=None, op0=ALU.max,
                    )
                    nc.vector.reciprocal(rcp[:, 1, :], rcp[:, 0, :])
                    x_sb = sp.tile([P, hsz], f32, tag="x_sb")
                    nc.vector.tensor_tensor(
                        out=x_sb[:].rearrange("p (h f) -> p h f", h=heads),
                        in0=ps[:, heads:nmsg].rearrange("p (h f) -> p h f", h=heads),
                        in1=rcp[:, 1, :].unsqueeze(2).to_broadcast([P, heads, F]),
                        op=ALU.mult,
                    )
                    if nxt != "out":
                        nc.vector.tensor_scalar_max(x_sb[:], x_sb[:], 0.0)
                    if nxt == "out":
                        nc.sync.dma_start(
                            out_ext[j * P : (j + 1) * P, :], x_sb[:, 0:F]
                        )
                    else:
                        wall_k, nw, tab_sh_n, row_n, hsz_n = nxt
                        dense_write(x_sb[:], j, wall_k, nw, tab_sh_n, row_n, hsz_n, None)
                    if next_ag is not None and j in CHUNK_END:
                        ag_chunk(next_ag[0], next_ag[1], CHUNK_END[j])

            if PHASES == 2:
                dump_rows(tab1_sh, ROW, D)
            if PHASES == 3:
                edge_layer(tab1, tab1_sh, ROW, H, D, "out", None)
            if PHASES >= 4:
                edge_layer(
                    tab1, tab1_sh, ROW, H, D,
                    (wall2_t, NW, tab2_sh, ROW, D),
                    (tab2_sh, tab2) if PHASES >= 5 else None,
                )
            if PHASES == 4:
                dump_rows(tab2_sh, ROW, D)
            if PHASES >= 5:
                edge_layer(
                    tab2, tab2_sh, ROW, H, D,
                    (wall3_t, NW3, tab3_sh, ROW3, F),
                    (tab3_sh, tab3),
                )
                edge_layer(tab3, tab3_sh, ROW3, 1, F, "out", None)

    nc.compile()

    in_maps = [
        {
            "cblob": blob_for_core(c),
            "featT": np.ascontiguousarray(featT[:, c * SHARD : (c + 1) * SHARD]),
            "selt": selt_for_core(c),
        }
        for c in range(NCORES)
    ]
    trace = os.environ.get("GAT_TRACE", "0") == "1"
    if trace and "antenv.axon_hooks" not in sys.modules:
        import types

        from trn_agent_boot.trn_boot import _ntff_profile_via_ctypes

        _hook = _ntff_profile_via_ctypes("/opt/axon/libaxon_pjrt.so")
        _mod = types.ModuleType("antenv.axon_hooks")
        _mod.get_axon_ntff_profile_hook = lambda: _hook
        _mod.set_axon_ntff_profile_hook = lambda h: None
        sys.modules["antenv.axon_hooks"] = _mod
    res = None
    for attempt in range(4):
        try:
            res = run_bass_kernel_spmd(
                nc, in_maps, list(range(NCORES)), trace=trace and attempt < 2
            )
            break
        except Exception:
            if attempt == 3:
                raise
            import time

            time.sleep(20 * (attempt + 1))
    if trace:
        print(f"HW exec time: {res.exec_time_ns} ns")
        global LAST_RESULTS
        LAST_RESULTS = res
    out = np.concatenate([res.results[c]["out"] for c in range(NCORES)], axis=0)
    return np.ascontiguousarray(out[:N]).astype(np.float32)


# revision 29
# speedup vs baseline: 1.1361x; 1.1361x over previous
"""3-layer GAT on 8 trn2 NeuronCores (Bass/Tile).

Strategy (dst-node sharding):
- N padded to 50176 = 392 dst-blocks x 128; 49 blocks per core.
- Per layer, a node-feature table holds per-node rows [h bf16 | el f32 | er f32]
  (768 B rows for 256-dim layers, 256 B rows for the 64-dim layer-3 input).
- Table rows use a chunk-interleaved layout: local blocks are grouped into
  chunks of CHK=7; chunk k of all 8 cores is contiguous (core-major inside
  the chunk).  That makes each chunk's AllGather output contiguous, so the
  three table AllGathers are split into 7 chunk-AGs fired as soon as their
  shard rows are written -- they overlap the dense/edge compute instead of
  serializing between layers.
- Dense phase is sharded: each core computes rows for its 6272 nodes (fp32
  matmuls on PE), writing its shard; chunk-AGs replicate to every core.
- Edge phase per dst-block: dma_gather pulls h|el rows by (remapped) src
  (two gathers, trow < 32768 and >= 32768, because gather indices are
  int16).  Gather descriptor generation on gpsimd is the kernel bottleneck
  (~8 ns/row), so the index arrays are -1-padded and the true per-block
  counts are loaded into gpsimd registers at runtime (reg_load) so the
  ucode only generates descriptors for real edges.
- er expansion per edge uses a host-precomputed transposed one-hot selT
  (bf16, streamed per block from HBM) as matmul lhsT -- no PE transposes or
  PSUM round-trips.  The dst one-hot sel is built on DVE and used to
  segment-reduce [w | w*h] into PSUM over the block's edge tiles, yielding
  softmax denominator and weighted sum together: out = (sum w*h)/(sum w).
- The next layer's dense matmul for the block's 128 nodes is interleaved
  right after each block epilogue so it hides inside the gather stream.
"""

import os
import sys

sys.path.insert(0, "/opt/trn_rl_repo")

PHASES = int(os.environ.get("GAT_PHASES", "99"))
EDGE_CUT = int(os.environ.get("GAT_EDGE_CUT", "99"))
EXACT = int(os.environ.get("GAT_EXACT", "0"))  # -1-padded gathers + runtime counts
NOCHUNK = int(os.environ.get("GAT_NOCHUNK", "0"))  # AllGathers after loops, no overlap
SINGLE_PACKET = bool(int(os.environ.get("GAT_SP", "0")))  # dma_gather single_packet

import numpy as np
import ml_dtypes

import concourse.bass as bass
import concourse.tile as tile
import concourse.mybir as mybir
from concourse import bacc
from concourse.bass_utils import run_bass_kernel_spmd

bf16 = mybir.dt.bfloat16
f32 = mybir.dt.float32
i16 = mybir.dt.int16
i32 = mybir.dt.int32
AF = mybir.ActivationFunctionType
ALU = mybir.AluOpType

NCORES = 8
P = 128
SPLIT = 32768
NEG_SLOPE = 0.2
H = 4
F = 64
D = H * F  # 256
ROW = 384  # bf16 cols per 256-dim table row (h 0:256 | el f32 256:264 | er f32 264:272 | pad)
ROW3 = 128  # bf16 cols per 64-dim table row (h 0:64 | el f32 64:66 | er f32 66:68 | pad)
CHK = 7  # blocks per AllGather chunk


def _wrap_idx_blocks(arr):
    """[NBLK, K] int16 -> [128, NBLK*K//16] in dma_gather index layout
    (idx i of each block at partition i%16, col i//16; 16-row pattern tiled
    8x down the partitions)."""
    nblk, k = arr.shape
    a = arr.reshape(nblk, k // 16, 16).transpose(0, 2, 1)  # [NBLK, 16, K/16]
    a = np.tile(a, (1, 8, 1))  # [NBLK, 128, K/16]
    return np.ascontiguousarray(a.transpose(1, 0, 2).reshape(128, -1))


def _col_layout(arr):
    """[NBLK, T*128] -> [128, NBLK*T]: slot t*128+p of block b at
    (p, b*T + t) -- matches the gather tile layout."""
    nblk, tk = arr.shape
    t = tk // 128
    a = arr.reshape(nblk, t, 128).transpose(2, 0, 1)  # [128, NBLK, T]
    return np.ascontiguousarray(a.reshape(128, nblk * t))


def _block_diag(a):
    """[H, F] -> [H*F, H] with a[h] on block-column h."""
    h, f = a.shape
    out = np.zeros((h * f, h), np.float32)
    for i in range(h):
        out[i * f : (i + 1) * f, i] = a[i]
    return out


def kernel(feat, src, dst, W1, al1, ar1, b1, W2, al2, ar2, b2, W3, al3, ar3, b3):
    feat = np.asarray(feat, np.float32)
    src = np.asarray(src).astype(np.int64)
    dst = np.asarray(dst).astype(np.int64)
    params = [np.asarray(p, np.float32) for p in (W1, al1, ar1, b1, W2, al2, ar2, b2, W3, al3, ar3, b3)]
    W1, al1, ar1, b1, W2, al2, ar2, b2, W3, al3, ar3, b3 = params
    assert abs(b1).max() == 0 and abs(b2).max() == 0 and abs(b3).max() == 0, (
        "non-zero GAT biases not implemented"
    )

    N, DIN = feat.shape
    E = src.shape[0]
    nblk_raw = -(-N // P)
    NBLK = -(-nblk_raw // NCORES) * NCORES  # 392
    NPAD = NBLK * P  # 50176
    BPC = NBLK // NCORES  # 49
    SHARD = BPC * P  # 6272
    # AllGather chunks over local blocks; small final chunk so the exposed
    # AG tail at each layer transition is tiny.
    CH = [7, 7, 7, 7, 7, 7, 4, 3]
    assert sum(CH) == BPC
    NCHUNK = len(CH)
    CS = np.concatenate([[0], np.cumsum(CH)])  # chunk start blocks, len NCHUNK+1

    # ---- host: edge preprocessing ----
    # table-row remap: node (core c, local block b, lane p) lives at
    # trow = 8*128*CS[k] + c*(CH[k]*128) + (b-CS[k])*128 + p  (k = chunk of b)
    # so each chunk's AllGather output is contiguous (core-major in-chunk).
    def trow_of(node):
        c = node // SHARD
        r = node % SHARD
        b = r // P
        p = r % P
        k = np.searchsorted(CS, b, side="right") - 1
        return NCORES * P * CS[k] + c * (np.asarray(CH)[k] * P) + (b - CS[k]) * P + p

    blk = dst // P
    order = np.lexsort((src, blk))
    tsrc_s = trow_of(src[order])
    dloc_s = (dst - blk * P)[order]
    blk_s = blk[order]
    counts = np.bincount(blk_s, minlength=NBLK)
    bstart = np.zeros(NBLK + 1, np.int64)
    np.cumsum(counts, out=bstart[1:])

    nlo = np.empty(NBLK, np.int64)
    nhi = np.empty(NBLK, np.int64)
    lo_list = []
    hi_list = []
    dlo_list = []
    dhi_list = []
    for b in range(NBLK):
        s, e = bstart[b], bstart[b + 1]
        ts = tsrc_s[s:e]
        dd = dloc_s[s:e]
        m = ts < SPLIT
        lo_list.append(ts[m])
        hi_list.append(ts[~m] - SPLIT)
        dlo_list.append(dd[m])
        dhi_list.append(dd[~m])
        nlo[b] = int(m.sum())
        nhi[b] = int((e - s) - nlo[b])

    # per-LOCAL-block gather sizes: max over the 8 cores (SPMD shares the
    # program, but block j's K need only cover the 8 cores' counts, not the
    # global max) -- saves ~10% of gather descriptors.
    nlo_c = nlo.reshape(NCORES, BPC)
    nhi_c = nhi.reshape(NCORES, BPC)
    KLO_J = (-(-nlo_c.max(axis=0) // P) * P).astype(np.int64)  # [BPC]
    KHI_J = (-(-nhi_c.max(axis=0) // P) * P).astype(np.int64)
    if int(os.environ.get("GAT_UNIK", "0")):  # uniform (global-max) gather sizes
        KLO_J[:] = KLO_J.max()
        KHI_J[:] = KHI_J.max()
    TLO_J = KLO_J // P
    T_J = (KLO_J + KHI_J) // P
    TM = int(T_J.max())
    LO_OFF = np.concatenate([[0], np.cumsum(KLO_J // 16)])
    HI_OFF = np.concatenate([[0], np.cumsum(KHI_J // 16)])
    DL_OFF = np.concatenate([[0], np.cumsum(T_J)])
    SELT_OFF = np.concatenate([[0], np.cumsum(T_J * P)])

    pad_idx = -1 if EXACT else 0
    lo_idx = []  # per global block, [K_LO_j] int16
    hi_idx = []
    dstloc = []  # per global block, [T_j*128] f32
    for b in range(NBLK):
        j = b % BPC
        nl, nh = int(nlo[b]), int(nhi[b])
        li = np.full(int(KLO_J[j]), pad_idx, np.int16)
        hi_ = np.full(int(KHI_J[j]), pad_idx, np.int16)
        dl = np.full(int(T_J[j]) * P, -1.0, np.float32)
        li[:nl] = lo_list[b]
        hi_[:nh] = hi_list[b]
        dl[:nl] = dlo_list[b]
        dl[int(KLO_J[j]) : int(KLO_J[j]) + nh] = dhi_list[b]
        lo_idx.append(li)
        hi_idx.append(hi_)
        dstloc.append(dl)

    cnts = np.stack([nlo, nhi], axis=1).astype(np.int32)  # [NBLK, 2]

    # ---- host: weights ----
    def wall(W, al, ar):
        wel = W @ _block_diag(al)
        wer = W @ _block_diag(ar)
        return np.concatenate([W, wel, wer], axis=1).astype(np.float32)

    wall1 = wall(W1, al1, ar1)  # [DIN, 264]
    wall2 = wall(W2, al2, ar2)  # [256, 264]
    wall3 = wall(W3, al3, ar3)  # [256, 66]
    NW = D + 2 * H  # 264
    NW3 = F + 2  # 66

    featT = np.zeros((DIN, NPAD), np.float32)
    featT[:, :N] = feat.T

    iota_np = np.tile(np.arange(P, dtype=np.float32), (P, 1)).astype(ml_dtypes.bfloat16)
    idn_np = np.eye(P, dtype=np.float32)

    # ---- host: per-core const blob (single int16 tensor -> one DMA) ----
    wall1b = wall1.astype(ml_dtypes.bfloat16)  # bf16 copy for fast dense-1

    def blob_for_core(c):
        b0 = c * BPC
        fields = [
            iota_np.view(np.int16),  # 128 cols bf16
            idn_np.view(np.int16),  # 256 cols f32
            wall1b.view(np.int16),  # [DIN, 264] bf16
            wall2[0:P].view(np.int16),
            wall2[P : 2 * P].view(np.int16),
            wall3[0:P].view(np.int16),
            wall3[P : 2 * P].view(np.int16),
        ]
        fields += [_wrap_idx_blocks(lo_idx[b0 + j][None, :]) for j in range(BPC)]
        fields += [_wrap_idx_blocks(hi_idx[b0 + j][None, :]) for j in range(BPC)]
        fields += [np.tile(cnts[b0 : b0 + BPC].view(np.int16).reshape(1, -1), (P, 1))]
        fields += [
            _col_layout(dstloc[b0 + j].astype(ml_dtypes.bfloat16).view(np.int16)[None, :])
            for j in range(BPC)
        ]
        for f_ in fields:
            assert f_.shape[0] == P, f_.shape
        blob = np.concatenate(fields, axis=1)
        if blob.shape[1] % 2:
            blob = np.concatenate([blob, np.zeros((P, 1), np.int16)], axis=1)
        return np.ascontiguousarray(blob)

    def selt_for_core(c):
        """Transposed dst one-hot: [128 (dstnode q), sum_j T_j*128] bf16
        viewed as int16; col (j, t*128+p) is 1.0 iff dstloc[j][t*128+p]==q."""
        b0 = c * BPC
        d = np.concatenate([dstloc[b0 + j] for j in range(BPC)]).reshape(1, -1)
        q = np.arange(P, dtype=np.float32).reshape(P, 1)
        sel = (d == q).astype(ml_dtypes.bfloat16)
        return np.ascontiguousarray(sel.view(np.int16))

    assert DIN == P, "layer-1 input dim must be 128"
    offs = {}
    o = 0
    for name, w in [
        ("iota", 128),
        ("idn", 256),
        ("wall1b", NW),
        ("wall2k0", 2 * NW),
        ("wall2k1", 2 * NW),
        ("wall3k0", 2 * NW3),
        ("wall3k1", 2 * NW3),
        ("lo", int(LO_OFF[-1])),
        ("hi", int(HI_OFF[-1])),
        ("cnts", BPC * 4),
        ("dstloc", int(DL_OFF[-1])),
    ]:
        offs[name] = o
        o += w
    blob0 = blob_for_core(0)
    CB = blob0.shape[1]
    assert o == CB or o + 1 == CB, (o, CB)
    assert offs["cnts"] % 2 == 0, "int32 counts need even int16 offset"
    NSELT = int(SELT_OFF[-1])

    # ---- build program (identical for all cores; per-core data via inputs) ----
    nc = bacc.Bacc("TRN2", target_bir_lowering=False, debug=False, num_devices=NCORES)

    cblob_in = nc.dram_tensor("cblob", [P, CB], i16, kind="ExternalInput")
    featT_in = nc.dram_tensor("featT", [P, SHARD], f32, kind="ExternalInput")
    selt_in = nc.dram_tensor("selt", [P, NSELT], i16, kind="ExternalInput")
    out_ext = nc.dram_tensor("out", [SHARD, F], f32, kind="ExternalOutput")

    tab1_sh = nc.dram_tensor("tab1_sh", [SHARD, ROW], bf16)
    tab2_sh = nc.dram_tensor("tab2_sh", [SHARD, ROW], bf16)
    tab3_sh = nc.dram_tensor("tab3_sh", [SHARD, ROW3], bf16)
    tab1 = nc.dram_tensor("tab1", [NPAD, ROW], bf16, addr_space="Shared")
    tab2 = nc.dram_tensor("tab2", [NPAD, ROW], bf16, addr_space="Shared")
    tab3 = nc.dram_tensor("tab3", [NPAD, ROW3], bf16, addr_space="Shared")

    rg = [list(range(NCORES))]

    with tile.TileContext(nc) as tc:
        with (
            tc.tile_pool(name="const", bufs=1) as cp,
            tc.tile_pool(name="work", bufs=3) as wp,
            tc.tile_pool(name="small", bufs=3) as sp,
            tc.tile_pool(name="psum", bufs=2, space="PSUM") as pp,
        ):
            cblob = cp.tile([P, CB], i16)
            nc.sync.dma_start(cblob[:], cblob_in[:])
            iota = cblob[:, offs["iota"] : offs["iota"] + 128].bitcast(bf16)
            idn = cblob[:, offs["idn"] : offs["idn"] + 256].bitcast(f32)
            wall1_t = cblob[:, offs["wall1b"] : offs["wall1b"] + NW].bitcast(bf16)
            wall2_t = [
                cblob[:, offs[f"wall2k{k}"] : offs[f"wall2k{k}"] + 2 * NW].bitcast(f32)
                for k in range(2)
            ]
            wall3_t = [
                cblob[:, offs[f"wall3k{k}"] : offs[f"wall3k{k}"] + 2 * NW3].bitcast(f32)
                for k in range(2)
            ]

            rcnt_lo = nc.gpsimd.alloc_register("cnt_lo")
            rcnt_hi = nc.gpsimd.alloc_register("cnt_hi")
            kregs = {
                int(v): nc.gpsimd.to_reg(int(v))
                for v in sorted(set(KLO_J.tolist()) | set(KHI_J.tolist()))
            }

            def idx_ap(field, joff, k16):
                off = offs[field] + joff
                return cblob[:, off : off + k16]

            def cnt_ap(j, which):
                off = offs["cnts"] + j * 4 + which * 2
                return cblob[0:1, off : off + 2].bitcast(i32)

            CHUNK_END = {int(CS[k + 1]) - 1: k for k in range(NCHUNK)}

            def ag_chunk_one(src_t, dst_t, k):
                lo, hi = int(CS[k]) * P, int(CS[k + 1]) * P
                nc.gpsimd.collective_compute(
                    "AllGather",
                    ALU.bypass,
                    replica_groups=rg,
                    ins=[src_t[lo:hi]],
                    outs=[dst_t[NCORES * lo : NCORES * hi]],
                )

            def ag_chunk(src_t, dst_t, k):
                if NOCHUNK:
                    if k == NCHUNK - 1:  # all chunks after the loop, no overlap
                        for kk in range(NCHUNK):
                            ag_chunk_one(src_t, dst_t, kk)
                    return
                ag_chunk_one(src_t, dst_t, k)

            def dense_write(x_ap, j, wall_k, nw, tab_shard, row_cols, hsz, first):
                """dense for 128 nodes of block j: rows [h bf16 | el er f32]
                written to tab_shard. x_ap: [128, 256] f32 node-major (SBUF),
                or None with `first` giving the layer-1 lhsT directly."""
                psd = pp.tile([P, NW], f32, tag="psd", space="PSUM")
                nk = len(wall_k)
                if first is not None:
                    nc.tensor.matmul(psd[:, :nw], first, wall_k[0][:, :nw], start=True, stop=True)
                else:
                    lhsT = sp.tile([P, 2, P], f32, tag="lhsT")
                    for k in range(nk):
                        ptr = pp.tile([P, P], f32, tag="ptr", space="PSUM")
                        nc.tensor.transpose(ptr[:], x_ap[:, k * P : (k + 1) * P], idn)
                        nc.vector.tensor_copy(lhsT[:, k, :], ptr[:])
                    for k in range(nk):
                        nc.tensor.matmul(
                            psd[:, :nw],
                            lhsT[:, k, :],
                            wall_k[k][:, :nw],
                            start=(k == 0),
                            stop=(k == nk - 1),
                        )
                row = sp.tile([P, row_cols], bf16, tag="row")
                nc.vector.tensor_copy(row[:, 0:hsz], psd[:, 0:hsz])
                nc.vector.tensor_copy(
                    row[:, hsz : hsz + 2 * (nw - hsz)].bitcast(f32),
                    psd[:, hsz:nw],
                )
                nc.sync.dma_start(tab_shard[j * P : (j + 1) * P, :], row[:])

            def dump_rows(tab_shard, row, hsz):
                """debug: write first 64 h-cols of each shard row to out_ext"""
                for j in range(BPC):
                    r = sp.tile([P, row], bf16, tag="dump")
                    nc.sync.dma_start(r[:], tab_shard[j * P : (j + 1) * P, :])
                    rf = sp.tile([P, F], f32, tag="dumpf")
                    nc.vector.tensor_copy(rf[:], r[:, 0:F])
                    nc.sync.dma_start(out_ext[j * P : (j + 1) * P, :], rf[:])

            # ---- dense layer 1 (sharded; lhsT = feat^T slices, K=128, bf16) ----
            for j in range(BPC):
                ft = sp.tile([P, P], f32, tag="ft")
                nc.sync.dma_start(ft[:], featT_in[:, j * P : (j + 1) * P])
                ftb = sp.tile([P, P], bf16, tag="ftb")
                nc.vector.tensor_copy(ftb[:], ft[:])
                dense_write(None, j, [wall1_t], NW, tab1_sh, ROW, D, first=ftb[:])
                if PHASES >= 2 and j in CHUNK_END:
                    ag_chunk(tab1_sh, tab1, CHUNK_END[j])

            if PHASES == 1:
                dump_rows(tab1_sh, ROW, D)

            # ---- edge phase for one layer ----
            def edge_layer(tab_full, tab_shard, row, heads, hsz, nxt, next_ag):
                """tab_full: AG'd table, tab_shard: local shard (er source),
                row: bf16 cols per table row, heads: H, hsz: h cols,
                nxt: (wall_k, nw, tab_shard_next, row_next, hsz_next) or
                'out' for the final layer.  next_ag: (shard_t, full_t) to
                chunk-AllGather as next-layer rows complete, or None."""
                nmsg = heads + hsz
                # zero the hx ring so stale SBUF bits can't turn into
                # NaN/inf messages on slots the shortened gathers skip
                for _ in range(3):
                    hx0 = wp.tile([P, TM, row], bf16, tag="hx")
                    nc.gpsimd.memset(hx0[:].rearrange("p t r -> p (t r)"), 0.0)
                for j in range(BPC):
                    klo, khi = int(KLO_J[j]), int(KHI_J[j])
                    tlo, tj = int(TLO_J[j]), int(T_J[j])
                    hx = wp.tile([P, TM, row], bf16, tag="hx")
                    if EXACT:
                        nc.gpsimd.reg_load(rcnt_lo, cnt_ap(j, 0))
                    nc.gpsimd.dma_gather(
                        hx[:, 0:tlo, :],
                        tab_full[0:SPLIT],
                        idx_ap("lo", int(LO_OFF[j]), klo // 16),
                        klo,
                        rcnt_lo if EXACT else kregs[klo],
                        row,
                        elem_step=row,
                        single_packet=SINGLE_PACKET,
                    )
                    if EXACT:
                        nc.gpsimd.reg_load(rcnt_hi, cnt_ap(j, 1))
                    nc.gpsimd.dma_gather(
                        hx[:, tlo:tj, :],
                        tab_full[SPLIT:NPAD],
                        idx_ap("hi", int(HI_OFF[j]), khi // 16),
                        khi,
                        rcnt_hi if EXACT else kregs[khi],
                        row,
                        elem_step=row,
                        single_packet=SINGLE_PACKET,
                    )
                    # er for the block's 128 dsts: direct strided load of the
                    # 256B [el|er] row chunk, cast er to bf16
                    erch = sp.tile([P, 128], bf16, tag="erch")
                    nc.sync.dma_start(
                        erch[:], tab_shard[j * P : (j + 1) * P, row - 128 : row]
                    )
                    eroff0 = 128 - (row - hsz)
                    er_blk = sp.tile([P, heads], bf16, tag="er_blk")
                    nc.scalar.activation(
                        er_blk[:],
                        erch[:, eroff0 + 2 * heads : eroff0 + 4 * heads].bitcast(f32),
                        AF.Copy,
                    )
                    if EDGE_CUT == 1:
                        # dump gathered h cols 0:64 of tile 0
                        df = sp.tile([P, F], f32, tag="edump")
                        nc.vector.tensor_copy(df[:], hx[:, 0, 0:F])
                        nc.sync.dma_start(out_ext[j * P : (j + 1) * P, :], df[:])
                        continue
                    # per-edge er via host-precomputed transposed one-hot
                    selt_sb = wp.tile([P, TM * P], i16, tag="selt")
                    so = int(SELT_OFF[j])
                    nc.sync.dma_start(
                        selt_sb[:, 0 : tj * P], selt_in[:, so : so + tj * P]
                    )
                    er_ps = pp.tile([P, TM * heads], f32, tag="erps", space="PSUM")
                    for t in range(tj):
                        nc.tensor.matmul(
                            er_ps[:, t * heads : (t + 1) * heads],
                            selt_sb[:, t * P : (t + 1) * P].bitcast(bf16),
                            er_blk[:],
                            start=True,
                            stop=True,
                        )
                    # dst one-hot for the segment matmul (DVE)
                    sel = wp.tile([P, TM, P], bf16, tag="sel")
                    dl_off = offs["dstloc"] + int(DL_OFF[j])
                    nc.vector.tensor_tensor(
                        out=sel[:, 0:tj, :],
                        in0=cblob[:, dl_off : dl_off + tj]
                        .bitcast(bf16)
                        .unsqueeze(2)
                        .to_broadcast([P, tj, P]),
                        in1=iota.unsqueeze(1).to_broadcast([P, tj, P]),
                        op=ALU.is_equal,
                    )
                    # e = el[src] + er[dst]; w = exp(lrelu(e))
                    # NOTE: elementwise ops run over the full TM extent so the
                    # access patterns stay tile-contiguous; tiles t >= tj hold
                    # junk (er_ps unwritten there) but the segment matmul only
                    # consumes t < tj, so the junk never reaches the output.
                    el_src = hx[:, :, hsz : hsz + 2 * heads].bitcast(f32)
                    e_t = sp.tile([P, TM, heads], f32, tag="e_t")
                    nc.vector.tensor_tensor(
                        out=e_t[:],
                        in0=el_src,
                        in1=er_ps[:].rearrange("p (t h) -> p t h", h=heads),
                        op=ALU.add,
                    )
                    lr = sp.tile([P, TM, heads], f32, tag="lr")
                    nc.vector.tensor_scalar_mul(lr[:], e_t[:], NEG_SLOPE)
                    nc.vector.tensor_tensor(out=lr[:], in0=e_t[:], in1=lr[:], op=ALU.max)
                    msg = wp.tile([P, TM, nmsg], bf16, tag="msg")
                    nc.scalar.activation(msg[:, :, 0:heads], lr[:], AF.Exp)
                    # wh = w * h
                    nc.vector.tensor_tensor(
                        out=msg[:, :, heads:nmsg],
                        in0=hx[:, :, 0:hsz],
                        in1=msg[:, :, 0:heads].unsqueeze(3).to_broadcast([P, TM, heads, F]),
                        op=ALU.mult,
                    )
                    if EDGE_CUT == 2:
                        df = sp.tile([P, F], f32, tag="edump")
                        nc.vector.tensor_copy(df[:], msg[:, 0, heads : heads + F])
                        nc.sync.dma_start(out_ext[j * P : (j + 1) * P, :], df[:])
                        continue
                    # segment-reduce into PSUM
                    ps = pp.tile([P, nmsg], f32, tag="agg", space="PSUM")
                    for t in range(tj):
                        nc.tensor.matmul(
                            ps[:],
                            sel[:, t, :],
                            msg[:, t, :],
                            start=(t == 0),
                            stop=(t == tj - 1),
                        )
                    if EDGE_CUT == 3:
                        df = sp.tile([P, F], f32, tag="edump")
                        nc.vector.tensor_copy(df[:], ps[:, heads : heads + F])
                        nc.sync.dma_start(out_ext[j * P : (j + 1) * P, :], df[:])
                        continue
                    if EDGE_CUT == 4:
                        df = sp.tile([P, F], f32, tag="edump")
                        nc.gpsimd.memset(df[:], 0.0)
                        nc.vector.tensor_copy(df[:, 0:heads], ps[:, 0:heads])
                        nc.sync.dma_start(out_ext[j * P : (j + 1) * P, :], df[:])
                        continue
                    # epilogue: out = act(wh_sum / w_sum)
                    rcp = sp.tile([P, 2, heads], f32, tag="rcp")
                    nc.vector.tensor_scalar(
                        out=rcp[:, 0, :], in0=ps[:, 0:heads], scalar1=1e-30,
                        scalar2=None, op0=ALU.max,
                    )
                    nc.vector.reciprocal(rcp[:, 1, :], rcp[:, 0, :])
                    x_sb = sp.tile([P, hsz], f32, tag="x_sb")
                    nc.vector.tensor_tensor(
                        out=x_sb[:].rearrange("p (h f) -> p h f", h=heads),
                        in0=ps[:, heads:nmsg].rearrange("p (h f) -> p h f", h=heads),
                        in1=rcp[:, 1, :].unsqueeze(2).to_broadcast([P, heads, F]),
                        op=ALU.mult,
                    )
                    if nxt != "out":
                        nc.vector.tensor_scalar_max(x_sb[:], x_sb[:], 0.0)
                    if nxt == "out":
                        nc.sync.dma_start(
                            out_ext[j * P : (j + 1) * P, :], x_sb[:, 0:F]
                        )
                    else:
                        wall_k, nw, tab_sh_n, row_n, hsz_n = nxt
                        dense_write(x_sb[:], j, wall_k, nw, tab_sh_n, row_n, hsz_n, None)
                    if next_ag is not None and j in CHUNK_END:
                        ag_chunk(next_ag[0], next_ag[1], CHUNK_END[j])

            if PHASES == 2:
                dump_rows(tab1_sh, ROW, D)
            if PHASES == 3:
                edge_layer(tab1, tab1_sh, ROW, H, D, "out", None)
            if PHASES >= 4:
                edge_layer(
                    tab1, tab1_sh, ROW, H, D,
                    (wall2_t, NW, tab2_sh, ROW, D),
                    (tab2_sh, tab2) if PHASES >= 5 else None,
                )
            if PHASES == 4:
                dump_rows(tab2_sh, ROW, D)
            if PHASES >= 5:
                edge_layer(
                    tab2, tab2_sh, ROW, H, D,
                    (wall3_t, NW3, tab3_sh, ROW3, F),
                    (tab3_sh, tab3),
                )
                edge_layer(tab3, tab3_sh, ROW3, 1, F, "out", None)

    nc.compile()

    in_maps = [
        {
            "cblob": blob_for_core(c),
            "featT": np.ascontiguousarray(featT[:, c * SHARD : (c + 1) * SHARD]),
            "selt": selt_for_core(c),
        }
        for c in range(NCORES)
    ]
    trace = os.environ.get("GAT_TRACE", "0") == "1"
    if trace and "antenv.axon_hooks" not in sys.modules:
        import types

        from trn_agent_boot.trn_boot import _ntff_profile_via_ctypes

        _hook = _ntff_profile_via_ctypes("/opt/axon/libaxon_pjrt.so")
        _mod = types.ModuleType("antenv.axon_hooks")
        _mod.get_axon_ntff_profile_hook = lambda: _hook
        _mod.set_axon_ntff_profile_hook = lambda h: None
        sys.modules["antenv.axon_hooks"] = _mod
    res = None
    for attempt in range(4):
        try:
            res = run_bass_kernel_spmd(
                nc, in_maps, list(range(NCORES)), trace=trace and attempt < 2
            )
            break
        except Exception:
            if attempt == 3:
                raise
            import time

            time.sleep(20 * (attempt + 1))
    if trace:
        print(f"HW exec time: {res.exec_time_ns} ns")
        global LAST_RESULTS
        LAST_RESULTS = res
    out = np.concatenate([res.results[c]["out"] for c in range(NCORES)], axis=0)
    return np.ascontiguousarray(out[:N]).astype(np.float32)
